# revision 12
# baseline (speedup 1.0000x reference)
"""Trainium2 Bass kernel for a custom LSTM cell.

Math (per reference):
    i = sigmoid(x @ W_i.T + b_Wi + h @ U_i.T + b_Ui)
    f = sigmoid(x @ W_f.T + b_Wf + h @ U_f.T + b_Uf + boundary @ W_b.T + b_Wb)
    o = sigmoid(x @ W_o.T + b_Wo + h @ U_o.T + b_Uo)
    g = tanh   (x @ W_g.T + b_Wg + h @ U_g.T + b_Ug)
    c = f * c_prev + i * g
    h = o * tanh(c)

Strategy: data-parallel over batch across 8 NeuronCores (1024 rows each),
feature-major ("transposed") layout so gate outputs land with the hidden
feature on the partition axis. Per core we compute

    pre[feat, batch] = M.T[feat, k] @ A[k, batch]

with A = [x | h_prev].T (K=1536) and M the fused [1536, 4096] weight
matrix, columns grouped per 128-wide h-slice as [i | f | o | g]. The
per-feature bias rides along the partition axis, so it is fused into the
sigmoid/tanh activation instruction (bias operand); the rank-2 boundary
term (f gate only) is evaluated off the PE as two scalar_tensor_tensor
ops on the DVE (outer product via per-partition scalar), so
the PE stream is pure 128x128x512 bf16 matmuls at full rate. All matmul
operands are bf16 (full PE rate, half the HBM traffic of fp32);
accumulation is fp32 in PSUM, end-to-end relative error ~4e-3.
"""

import sys

sys.path.insert(0, "/opt/trn_rl_repo")

import numpy as np
import ml_dtypes

B, IN, H = 8192, 512, 1024
NCORES = 8
BLOC = B // NCORES  # 1024 rows per core
KTOT = IN + H  # 1536 contraction
KT = KTOT // 128  # 12 k-tiles
NSL = H // 128  # 8 h-slices of 128 features
NBH = 2  # batch halves of 512
NWU = 8  # PE warm-up matmuls (N=512 each)

_PROG = None  # cached so repeat calls skip rebuild/recompile


def _build_program():
    import concourse.bass as bass
    import concourse.mybir as mybir
    import concourse.tile as tile
    from concourse import bacc
    from contextlib import ExitStack

    f32 = mybir.dt.float32
    bf16 = mybir.dt.bfloat16
    SIG = mybir.ActivationFunctionType.Sigmoid
    TANH = mybir.ActivationFunctionType.Tanh
    MULT = mybir.AluOpType.mult
    ADD = mybir.AluOpType.add

    nc = bacc.Bacc("TRN2", target_bir_lowering=False, debug=False)

    at_d = nc.dram_tensor("at_in", [KTOT, BLOC], bf16, kind="ExternalInput").ap()
    m_d = nc.dram_tensor("m_in", [KTOT, 4 * H], bf16, kind="ExternalInput").ap()
    # boundary rows broadcast across 128 partitions, [2][128, BLOC]
    btb_d = nc.dram_tensor("btb_in", [2, 128, BLOC], bf16, kind="ExternalInput").ap()
    # W_b columns packed per-feature: wbp[p, 2*s+r] = W_b[s*128+p, r]
    wbp_d = nc.dram_tensor("wbp_in", [128, 2 * NSL], f32, kind="ExternalInput").ap()
    bias_d = nc.dram_tensor("bias_in", [128, 4 * NSL], f32, kind="ExternalInput").ap()
    ct_d = nc.dram_tensor("ct_in", [H, BLOC], bf16, kind="ExternalInput").ap()
    h_o = nc.dram_tensor("h_out", [H, BLOC], bf16, kind="ExternalOutput").ap()
    c_o = nc.dram_tensor("c_out", [H, BLOC], bf16, kind="ExternalOutput").ap()

    with tile.TileContext(nc) as tc:
        with ExitStack() as ctx:
            cst = ctx.enter_context(tc.tile_pool(name="cst", bufs=1))
            atp = ctx.enter_context(tc.tile_pool(name="atp", bufs=1))
            mp = ctx.enter_context(tc.tile_pool(name="mp", bufs=1))
            ctp = ctx.enter_context(tc.tile_pool(name="ctp", bufs=1))
            actp = ctx.enter_context(tc.tile_pool(name="actp", bufs=2))
            prep = ctx.enter_context(tc.tile_pool(name="prep", bufs=2))
            outp = ctx.enter_context(tc.tile_pool(name="outp", bufs=4))
            psp = ctx.enter_context(tc.tile_pool(name="psp", bufs=8, space="PSUM"))
            wup = ctx.enter_context(tc.tile_pool(name="wup", bufs=1))

            # PE warm-up: dummy bf16 matmuls with no DMA deps keep the PE
            # clocked up while the first weight tiles load. One accumulation
            # chain so they pipeline back-to-back with no bank-reset stalls.
            wu_w = wup.tile([128, 128], bf16, name="wu_w")
            nc.vector.memset(wu_w, 0.0)
            wu_r = wup.tile([128, 512], bf16, name="wu_r")
            nc.vector.memset(wu_r, 0.0)
            wu_ps = psp.tile([128, 512], f32, name="wu_ps", tag="ps")
            for j in range(NWU):
                nc.tensor.matmul(
                    wu_ps, wu_w, wu_r, start=(j == 0), stop=(j == NWU - 1)
                )

            # Small constants + c_prev.T slices ride the Activation-engine DMA
            # queue (outputs come much later), away from the big weight stream.
            bias_t = cst.tile([128, 4 * NSL], f32, name="bias_t")
            nc.scalar.dma_start(out=bias_t, in_=bias_d[:, :])
            wbp_t = cst.tile([128, 2 * NSL], f32, name="wbp_t")
            nc.scalar.dma_start(out=wbp_t, in_=wbp_d[:, :])
            btb_t = cst.tile([128, 2, BLOC], bf16, name="btb_t")
            nc.scalar.dma_start(out=btb_t[:, 0, :], in_=btb_d[0, :, :])
            nc.scalar.dma_start(out=btb_t[:, 1, :], in_=btb_d[1, :, :])

            ct_t = ctp.tile([128, NSL, BLOC], bf16, name="ct_t")
            for s in range(NSL):
                nc.scalar.dma_start(
                    out=ct_t[:, s, :], in_=ct_d[s * 128 : (s + 1) * 128, :]
                )

            # A.T [128, 12, 1024] interleaved with the m slice-groups it gates.
            at_t = atp.tile([128, KT, BLOC], bf16, name="at_t")
            m_t = mp.tile([128, KT, 4 * H], bf16, name="m_t")

            def load_at(eng, k0, k1, b0, b1):
                eng.dma_start(
                    out=at_t[:, k0:k1, b0:b1],
                    in_=at_d[k0 * 128 : k1 * 128, b0:b1].rearrange(
                        "(kk p) g -> p kk g", p=128
                    ),
                )

            def load_m(eng, sg, k0, k1):
                eng.dma_start(
                    out=m_t[:, k0:k1, sg * 512 : (sg + 1) * 512],
                    in_=m_d[
                        k0 * 128 : k1 * 128, sg * 512 : (sg + 1) * 512
                    ].rearrange("(kk p) g -> p kk g", p=128),
                )

            # batch-half 0 sweeps all slices first, so only at[:, :512] gates
            # the start; at batch-half 1 is needed only ~80us in. The at/m
            # streams ride two HWDGE queues (SP + GpSimd) so both DMA rings
            # ramp in parallel; the first k-tile is split off so the opening
            # matmul's dependency is ~0.2 MB, not 1 MB.
            load_at(nc.sync, 0, 1, 0, 512)
            load_m(nc.gpsimd, 0, 0, 1)
            load_at(nc.sync, 1, 4, 0, 512)
            load_m(nc.gpsimd, 0, 1, 4)
            for j in (1, 2):
                load_at(nc.sync, j * 4, (j + 1) * 4, 0, 512)
                load_m(nc.gpsimd, 0, j * 4, (j + 1) * 4)
            flip = 0
            for sg in range(1, NSL):
                for j in range(3):
                    load_m(nc.sync if flip else nc.gpsimd, sg, j * 4, (j + 1) * 4)
                    flip ^= 1
            for j in range(3):
                load_at(nc.sync if j % 2 else nc.gpsimd, j * 4, (j + 1) * 4, 512, 1024)

            for gi in range(NBH * NSL):
                bh, sg = divmod(gi, NSL)
                last_group = gi == NBH * NSL - 1
                c0 = sg * 512
                if True:
                    bs = slice(bh * 512, (bh + 1) * 512)
                    ps = [
                        psp.tile([128, 512], f32, name=f"ps{z}_{sg}_{bh}", tag="ps")
                        for z in range(4)
                    ]
                    if not last_group:
                        for k in range(KT):
                            rhs = at_t[:, k, bs]
                            for z in range(4):
                                nc.tensor.matmul(
                                    ps[z],
                                    m_t[:, k, c0 + z * 128 : c0 + (z + 1) * 128],
                                    rhs,
                                    start=(k == 0),
                                    stop=(k == KT - 1),
                                )
                    else:
                        # final group runs gate-major (f first, o last) so the
                        # f->boundary->sigmoid->c drain chain overlaps the
                        # remaining matmuls; only o's act+mul+store trail.
                        for z in (1, 0, 3, 2):
                            for k in range(KT):
                                nc.tensor.matmul(
                                    ps[z],
                                    m_t[:, k, c0 + z * 128 : c0 + (z + 1) * 128],
                                    at_t[:, k, bs],
                                    start=(k == 0),
                                    stop=(k == KT - 1),
                                )

                    # boundary (f gate only): rank-2 outer product folded in on
                    # the Pool engine: u = bt0*wb0 + ps_f ; u = bt1*wb1 + u
                    uf = prep.tile([128, 512], f32, name=f"uf{sg}_{bh}", tag="uf")
                    nc.vector.scalar_tensor_tensor(
                        uf,
                        btb_t[:, 0, bs],
                        wbp_t[:, 2 * sg : 2 * sg + 1],
                        ps[1],
                        op0=MULT,
                        op1=ADD,
                    )
                    nc.vector.scalar_tensor_tensor(
                        uf,
                        btb_t[:, 1, bs],
                        wbp_t[:, 2 * sg + 1 : 2 * sg + 2],
                        uf,
                        op0=MULT,
                        op1=ADD,
                    )

                    # activations: bias is per-feature (partition axis) -> fused
                    ft = actp.tile([128, 512], bf16, name=f"f{sg}_{bh}", tag="f")
                    it = actp.tile([128, 512], bf16, name=f"i{sg}_{bh}", tag="i")
                    gt = actp.tile([128, 512], bf16, name=f"g{sg}_{bh}", tag="g")
                    ot = actp.tile([128, 512], bf16, name=f"o{sg}_{bh}", tag="o")
                    nc.scalar.activation(
                        ft, uf, SIG, bias=bias_t[:, sg * 4 + 1 : sg * 4 + 2]
                    )
                    nc.scalar.activation(
                        it, ps[0], SIG, bias=bias_t[:, sg * 4 + 0 : sg * 4 + 1]
                    )
                    nc.scalar.activation(
                        gt, ps[3], TANH, bias=bias_t[:, sg * 4 + 3 : sg * 4 + 4]
                    )
                    nc.scalar.activation(
                        ot, ps[2], SIG, bias=bias_t[:, sg * 4 + 2 : sg * 4 + 3]
                    )

                    cn = outp.tile([128, 512], bf16, name=f"cn{sg}_{bh}", tag="cn")
                    tmp = actp.tile([128, 512], bf16, name=f"tp{sg}_{bh}", tag="tp")
                    nc.vector.tensor_mul(cn, ft, ct_t[:, sg, bs])
                    nc.vector.tensor_mul(tmp, it, gt)
                    nc.vector.tensor_add(cn, cn, tmp)
                    th = actp.tile([128, 512], bf16, name=f"th{sg}_{bh}", tag="th")
                    nc.scalar.activation(th, cn, TANH)
                    hn = outp.tile([128, 512], bf16, name=f"hn{sg}_{bh}", tag="hn")
                    nc.vector.tensor_mul(hn, ot, th)

                    nc.scalar.dma_start(
                        out=c_o[sg * 128 : (sg + 1) * 128, bs], in_=cn
                    )
                    nc.scalar.dma_start(
                        out=h_o[sg * 128 : (sg + 1) * 128, bs], in_=hn
                    )
    nc.compile()
    return nc


def _get_program():
    global _PROG
    if _PROG is None:
        _PROG = _build_program()
    return _PROG


def _prep_inputs(inputs):
    """Host-side marshalling: fused transposed weights + bf16 casts."""
    f = np.float32
    bf = ml_dtypes.bfloat16
    x = np.asarray(inputs["x"], f)
    h_prev = np.asarray(inputs["h_prev"], f)
    c_prev = np.asarray(inputs["c_prev"], f)
    boundary = np.asarray(inputs["boundary"], f)

    gates = ["i", "f", "o", "g"]
    W = {z: np.asarray(inputs[f"W_{z}"], f) for z in gates}
    U = {z: np.asarray(inputs[f"U_{z}"], f) for z in gates}
    bias = {
        z: np.asarray(inputs[f"b_W{z}"], f) + np.asarray(inputs[f"b_U{z}"], f)
        for z in gates
    }
    W_b = np.asarray(inputs["W_b"], f)
    b_Wb = np.asarray(inputs["b_Wb"], f)
    bias["f"] = bias["f"] + b_Wb

    # M [1536, 4096]: rows 0-511 W.T, rows 512-1535 U.T; columns grouped per
    # 128-wide h-slice as [i | f | o | g].
    M = np.empty((KTOT, 4 * H), f)
    bias_pack = np.empty((128, 4 * NSL), f)
    wbp = np.empty((128, 2 * NSL), f)
    for s in range(NSL):
        hs = slice(s * 128, (s + 1) * 128)
        for zi, z in enumerate(gates):
            cs = slice(s * 512 + zi * 128, s * 512 + (zi + 1) * 128)
            M[:IN, cs] = W[z][hs].T
            M[IN:, cs] = U[z][hs].T
            bias_pack[:, s * 4 + zi] = bias[z][hs]
        wbp[:, 2 * s] = W_b[hs, 0]
        wbp[:, 2 * s + 1] = W_b[hs, 1]

    Mb = np.ascontiguousarray(M.astype(bf))
    AT = np.concatenate([x, h_prev], axis=1).T.astype(bf)  # [1536, 8192]
    BT = boundary.T.astype(bf)  # [2, 8192]
    CT = c_prev.T.astype(bf)  # [1024, 8192]

    in_maps = []
    for c in range(NCORES):
        rs = slice(c * BLOC, (c + 1) * BLOC)
        btb = np.broadcast_to(BT[:, None, rs], (2, 128, BLOC))
        in_maps.append(
            {
                "at_in": np.ascontiguousarray(AT[:, rs]),
                "m_in": Mb,
                "btb_in": np.ascontiguousarray(btb),
                "wbp_in": wbp,
                "bias_in": bias_pack,
                "ct_in": np.ascontiguousarray(CT[:, rs]),
            }
        )
    return in_maps


def run(inputs, trace=False):
    """Returns ((h, c), BassKernelResults)."""
    from concourse.bass_utils import run_bass_kernel_spmd

    nc = _get_program()
    in_maps = _prep_inputs(inputs)
    res = run_bass_kernel_spmd(
        nc, in_maps, core_ids=list(range(NCORES)), trace=trace
    )
    h = np.concatenate(
        [np.asarray(r["h_out"], np.float32).T for r in res.results], axis=0
    )
    c = np.concatenate(
        [np.asarray(r["c_out"], np.float32).T for r in res.results], axis=0
    )
    return (h, c), res


def kernel(**inputs):
    out, _ = run(inputs, trace=False)
    return out
